# revision 67
# baseline (speedup 1.0000x reference)
"""Causal self-attention (B=4, S=2048, D=768, H=12) on 8 TRN2 NeuronCores.

Sharding: core = (batch b in 0..3) x (head-group hg in 0..1, 6 heads each).
Host pre-transposes x -> xT per batch (bf16), slices w_qkv columns /
w_proj rows per head-group (bf16), reorders w_qkv columns pair-major
[q_p0|k_p0|q_p1|k_p1|q_p2|k_p2|v].  Each core computes its 6 heads
end-to-end and a partial projection output [S, D]; the host sums the two
head-group partials per batch and adds b_proj plus the (attention-
invariant) v-bias term b_v @ w_proj.  The k bias is dropped entirely:
softmax over k is invariant to the per-q constant (q+bq).bk.

Device design notes:
  - Every matmul's moving operand is BF16 (or f32r with >=256 free), so
    all matmuls run at 1 cycle/row; the PE stream is kept dense so the
    p-state ramps to 2.4 GHz and stays there.
  - Causal masking happens ON the PE: a [128,128] constant matmul
    (lhsT=maskT f32r, rhs=identity bf16) accumulates -240 into the
    upper-triangle of diagonal score blocks before exp(0.125*x).  No
    cross-engine op sits between scores and exp.
  - exp on ACT per (chunk,h) over [128,2,512] PSUM->SBUF bf16; the
    attn@v matmuls lag two chunks behind so ACT latency never stalls
    the in-order PE queue.
  - PSUM->SBUF drains (q bias-add, k copy, v scatter, yh copy, out
    copy) on DVE; normalization broadcast+multiply on Pool; DMA issues
    split across SP/Pool/ACT queues, biggest-stakeholder-first.
  - v carries a ones-column so attn@[v|1] also emits the softmax
    denominator row (yh partition 64).
"""

import numpy as np
from collections import deque
from contextlib import ExitStack

import concourse.bacc as bacc
import concourse.mybir as mybir
from concourse.tile import TileContext

F32 = mybir.dt.float32
F32R = mybir.dt.float32r
BF16 = mybir.dt.bfloat16

D = 768
NCORES = 8
SCALE = 0.125  # 1/sqrt(64)
MASKV = -240.0  # pre-scale mask add: exp(0.125 * -240) = e^-30 ~ 1e-13


def build_program(S=2048):
    NS = S // 512   # q strips
    NT = S // 128   # s tiles
    DT = D // 128   # d tiles (contraction)

    nc = bacc.Bacc()

    # All inputs ship partition-major-packed from the host so every DMA
    # descriptor is a multi-KB contiguous run (descriptor-rate bound
    # otherwise).  wqkv columns: [bias(1) | qp0 kp0 qp1 kp1 qp2 kp2 | v].
    xT = nc.dram_tensor("xT_s", [NS, 128, D // 128, 512], BF16,
                        kind="ExternalInput")
    wqkvA = nc.dram_tensor("wqkvA_s", [128, D // 128, 257], BF16,
                           kind="ExternalInput")
    wqkvB = nc.dram_tensor("wqkvB_s", [128, D // 128, 896], BF16,
                           kind="ExternalInput")
    wproj = nc.dram_tensor("wproj_s", [128, 3, D], BF16,
                           kind="ExternalInput")
    out = nc.dram_tensor("out_s", [S, D], F32, kind="ExternalOutput")

    with TileContext(nc) as tc, ExitStack() as ctx:
        persist = ctx.enter_context(tc.tile_pool(name="persist", bufs=1))

        w1a = persist.tile([128, 6, 257], BF16, tag="w1a", name="w1a")
        w1b = persist.tile([128, 6, 896], BF16, tag="w1b", name="w1b")
        xT1 = persist.tile([128, NS, 6, 512], BF16, tag="xT1", name="xT1")
        wp1 = persist.tile([128, 3, D], BF16, tag="wp1", name="wp1")
        qT = [persist.tile([128, S], BF16, tag=f"qT{i}", name=f"qT{i}")
              for i in range(3)]
        kT = [persist.tile([128, S], BF16, tag=f"kT{i}", name=f"kT{i}")
              for i in range(3)]
        v_sb = [persist.tile([128, 6, 65], BF16, tag=f"v{i}", name=f"v{i}")
                for i in range(NT)]
        yT = [persist.tile([128, S], BF16, tag=f"yT{i}", name=f"yT{i}")
              for i in range(3)]
        bq_sb = persist.tile([128, 3], F32, tag="bq", name="bq_sb")
        mskT_sb = persist.tile([128, 128], BF16, tag="mskT", name="mskT_sb")
        idf = persist.tile([128, 128], F32, tag="idf", name="idf")
        iden_sb = persist.tile([128, 128], BF16, tag="iden", name="iden_sb")
        mkf = persist.tile([128, 128], F32, tag="mkf", name="mkf")

        # identity: ones, zero j<p, zero j>p (affine_select keeps where
        # base + j + channel_multiplier*p  <cmp>  0), then cast to bf16
        nc.gpsimd.memset(idf[:], 1.0)
        nc.gpsimd.affine_select(
            out=idf[:], in_=idf[:],
            compare_op=mybir.AluOpType.is_ge, fill=0.0, base=0,
            pattern=[[1, 128]], channel_multiplier=-1)  # keep j >= p
        nc.gpsimd.affine_select(
            out=idf[:], in_=idf[:],
            compare_op=mybir.AluOpType.is_ge, fill=0.0, base=0,
            pattern=[[-1, 128]], channel_multiplier=1)  # keep p >= j
        nc.vector.tensor_copy(iden_sb[:], idf[:])
        # maskT[r, c] = MASKV where c > r else 0 (built on-device: no DMA
        # on the critical path to the first masked score block)
        nc.gpsimd.memset(mkf[:], 0.0)
        nc.gpsimd.affine_select(
            out=mkf[:], in_=mkf[:],
            compare_op=mybir.AluOpType.is_ge, fill=MASKV, base=0,
            pattern=[[-1, 128]], channel_multiplier=1)  # fill where c > r
        nc.vector.tensor_copy(mskT_sb[:], mkf[:])

        # ---- input DMAs: need-ordered; SP ring carries the first needs,
        # the Pool ring the rest.  All transfers are contiguous-per-
        # partition (multi-KB descriptors).
        nc.scalar.dma_start(out=w1a[:], in_=wqkvA[:])
        nc.sync.dma_start(out=xT1[:, 0], in_=xT[0])
        nc.sync.dma_start(out=w1b[:], in_=wqkvB[:])
        for ns2 in range(1, NS):
            nc.gpsimd.dma_start(out=xT1[:, ns2], in_=xT[ns2])
        nc.gpsimd.dma_start(out=wp1[:], in_=wproj[:])
        for st in range(NT):
            nc.gpsimd.memset(v_sb[st][:, :, 64:65], 1.0)
        # q bias rides in w1a column 0 (bf16); cast once to f32 for the
        # tensor_scalar_add scalar operand
        nc.vector.tensor_copy(
            bq_sb[:], w1a[:, 0:3, 0:1].rearrange("p t c -> p (t c)"))

        ps = ctx.enter_context(tc.tile_pool(name="ps", bufs=1, space="PSUM"))
        expp = ctx.enter_context(tc.tile_pool(name="expp", bufs=8))
        rcp = ctx.enter_context(tc.tile_pool(name="rcp", bufs=4))
        rbp = ctx.enter_context(tc.tile_pool(name="rbp", bufs=2))
        outp = ctx.enter_context(tc.tile_pool(name="outp", bufs=2))

        def pe_touch(ap):
            # Tiny self-matmul so the PE waits on this tile's producer
            # once; later matmuls using it carry at most one sync wait.
            t = ps.tile([1, 1], F32, tag="mm", bufs=2, name="touch")
            nc.tensor.matmul(t[:], ap, ap, start=True, stop=True)

        pe_touch(mskT_sb[:, 0:2].bitcast(F32))
        pe_touch(iden_sb[:, 0:2].bitcast(F32))

        # ---- phase work units (emitted interleaved) ----
        def p1q_unit(ns, p):
            # qT[p][strip ns] = (wq_p.T @ xT) + bq_p   (bf16 out)
            psu = ps.tile([128, 512], F32, tag="mm", bufs=2, name="ps_q")
            for dt_i in range(DT):
                wsl = (w1a[:, dt_i, 1:129] if p == 0 else
                       w1b[:, dt_i, 256 * p - 256:256 * p - 128])
                nc.tensor.matmul(
                    psu[:], wsl, xT1[:, ns, dt_i, :],
                    start=(dt_i == 0), stop=(dt_i == DT - 1))
            nc.vector.tensor_scalar_add(
                qT[p][:, 512 * ns:512 * ns + 512], psu[:],
                bq_sb[:, p:p + 1])

        def p1k_unit(ns, p):
            # kT[p][strip ns] = wk_p.T @ xT   (no bias; bf16 out)
            psu = ps.tile([128, 512], F32, tag="mm", bufs=2, name="ps_k")
            for dt_i in range(DT):
                wsl = (w1a[:, dt_i, 129:257] if p == 0 else
                       w1b[:, dt_i, 256 * p - 128:256 * p])
                nc.tensor.matmul(
                    psu[:], wsl, xT1[:, ns, dt_i, :],
                    start=(dt_i == 0), stop=(dt_i == DT - 1))
            nc.vector.tensor_copy(kT[p][:, 512 * ns:512 * ns + 512], psu[:])

        def p2v_unit(st):
            # v natural for s-tile st (bf16, ones in col 64)
            psu = ps.tile([128, 512], F32, tag="mm", bufs=2, name="ps_v")
            sq, so = st // 4, 128 * (st % 4)
            for dt_i in range(DT):
                nc.tensor.matmul(
                    psu[:, 0:384], xT1[:, sq, dt_i, so:so + 128],
                    w1b[:, dt_i, 512:896],
                    start=(dt_i == 0), stop=(dt_i == DT - 1))
            nc.vector.tensor_copy(
                v_sb[st][:, :, 0:64],
                psu[:, 0:384].rearrange("p (h e) -> p h e", h=6))

        def p4_unit(st, tail=False):
            # partial proj for s-tile st.  In the endgame (attention done)
            # the freed score PSUM banks + the ACT engine double the proj
            # drain pipeline depth.
            if tail:
                buf = ps.tile([128, 2, 512], F32, tag="sc", bufs=2,
                              name="pt")
                pa, pb = buf[:, 0, :], buf[:, 1, 0:256]
            else:
                pa = ps.tile([128, 512], F32, tag="mm", bufs=2, name="pa")[:]
                pb = ps.tile([128, 512], F32, tag="mm", bufs=2,
                             name="pb")[:, 0:256]
            for yt in range(3):
                nc.tensor.matmul(
                    pa, yT[yt][:, 128 * st:128 * st + 128],
                    wp1[:, yt, 0:512], start=(yt == 0), stop=(yt == 2))
            for yt in range(3):
                nc.tensor.matmul(
                    pb, yT[yt][:, 128 * st:128 * st + 128],
                    wp1[:, yt, 512:768], start=(yt == 0), stop=(yt == 2))
            ot = outp.tile([128, D], F32, tag="ot", name="ot")
            nc.vector.tensor_copy(ot[:, 0:512], pa)
            nc.vector.tensor_copy(ot[:, 512:768], pb)
            nc.sync.dma_start(out=out[128 * st:128 * st + 128, :], in_=ot[:])

        pre_q = deque()   # next strip's q/k/v units (due before that strip)
        opt_q = deque()   # proj units (any time after their strip + norm)
        OPT_KEEP = 5      # proj units held back to cover the final tail

        def drain(n):
            for _ in range(n):
                if pre_q:
                    pre_q.popleft()()
                elif len(opt_q) > OPT_KEEP:
                    p4_unit(opt_q.popleft())
                else:
                    return

        def drain_prereqs():
            while pre_q:
                pre_q.popleft()()

        # prologue: strip-0 pair-0 q/k and all strip-0 v before attention;
        # pairs 1,2 emit at their hp boundary (keeps PE fed while ACT ramps)
        p1q_unit(0, 0)
        p1k_unit(0, 0)
        for st in range(4 if NS > 1 else NT):
            p2v_unit(st)
        due_hp = {1: [1], 2: [2]}

        LAG = 2  # attn@v trails the exp pipeline by this many chunks

        # ---- attention ----
        for ns in range(NS):
            if ns + 1 < NS:
                for p in range(3):
                    pre_q.append(lambda a=ns + 1, b=p: p1q_unit(a, b))
                    pre_q.append(lambda a=ns + 1, b=p: p1k_unit(a, b))
                for st in range(4 * (ns + 1), min(4 * (ns + 2), NT)):
                    pre_q.append(lambda a=st: p2v_unit(a))
                if ns == 0:
                    pre_q.append(lambda: pe_touch(wp1[:, 0, 0:2].bitcast(F32)))
            q0 = 512 * ns
            for hp in range(3):
                if ns == 0:
                    for p in due_hp.pop(hp, []):
                        p1q_unit(0, p)
                        p1k_unit(0, p)
                nk = 4 * (ns + 1)
                nchunk = nk // 2
                yh = [ps.tile([65, 512], F32, tag="yh", bufs=2, name="yh0"),
                      ps.tile([65, 512], F32, tag="yh", bufs=2, name="yh1")]

                def emit_att(c, ex_pair):
                    # attn@v accumulation for chunk c (both heads, both u)
                    for h in range(2):
                        for u in range(2):
                            kb = 2 * c + u
                            c0 = max(0, 128 * kb - q0)
                            nc.tensor.matmul(
                                yh[h][:, c0:512],
                                v_sb[kb][:, 2 * hp + h, :],
                                ex_pair[h][:, u, c0:512],
                                start=(kb == 0), stop=(kb == nk - 1),
                                skip_group_check=True)

                pend = deque()
                for c in range(nchunk):
                    ex_pair = []
                    for h in range(2):
                        p0 = 64 * h
                        sc2 = ps.tile([128, 2, 512], F32, tag="sc", bufs=2,
                                      name="sc2")
                        e_lo = 512
                        for u in range(2):
                            kb = 2 * c + u
                            d = kb - 4 * ns
                            q_lo = 128 * d if d >= 0 else 0
                            e_lo = min(e_lo, q_lo)
                            nc.tensor.matmul(
                                sc2[:, u, q_lo:512],
                                kT[hp][p0:p0 + 64, 128 * kb:128 * kb + 128],
                                qT[hp][p0:p0 + 64, q0 + q_lo:q0 + 512],
                                start=True, stop=(d < 0),
                                skip_group_check=True)
                            if d >= 0:
                                nc.tensor.matmul(
                                    sc2[:, u, 128 * d:128 * d + 128],
                                    mskT_sb[:],
                                    iden_sb[:],
                                    start=False, stop=True,
                                    skip_group_check=True)
                        ex2 = expp.tile([128, 2, 512], BF16, tag="exp",
                                        name="ex2")
                        nc.scalar.activation(
                            ex2[:, :, e_lo:512], sc2[:, :, e_lo:512],
                            mybir.ActivationFunctionType.Exp, scale=SCALE)
                        ex_pair.append(ex2)
                    pend.append((c, ex_pair))
                    # front-load pre-strip fillers so their DVE drains land
                    # well before the next strip's scores need the tiles
                    if c >= 1 or (ns == 0 and hp == 0):
                        drain(2 if pre_q else 1)
                    if len(pend) > LAG:
                        emit_att(*pend.popleft())
                while pend:
                    emit_att(*pend.popleft())
                    drain(1)

                # tail: free yh banks and normalize yT.  Mid-kernel the yh
                # copies go first (they free the banks for the next head
                # pair); on the very last tail the reciprocal path leads so
                # the Pool broadcasts start as early as possible (the final
                # proj units' yt2 step waits on the mult).
                last_tail = (ns == NS - 1 and hp == 2)
                for h in range(2):
                    ys = yT[hp][64 * h:64 * h + 64, q0:q0 + 512]
                    lrow = rcp.tile([1, 512], F32, tag="lrow", name="lrow")
                    rec = rcp.tile([1, 512], F32, tag="rec", name="rec")
                    rb = rbp.tile([128, 512], F32, tag="rb", name="rb")
                    if last_tail:
                        nc.vector.tensor_copy(lrow[:], yh[h][64:65, :])
                        nc.vector.reciprocal_approx_fast(rec[:], lrow[:])
                        nc.gpsimd.partition_broadcast(rb[:], rec[:])
                        nc.vector.tensor_copy(ys, yh[h][0:64, :])
                    else:
                        nc.vector.tensor_copy(ys, yh[h][0:64, :])
                        nc.vector.tensor_copy(lrow[:], yh[h][64:65, :])
                        nc.vector.reciprocal_approx_fast(rec[:], lrow[:])
                        nc.gpsimd.partition_broadcast(rb[:], rec[:])
                    nc.vector.tensor_mul(ys, ys, rb[64 * h:64 * h + 64, :])
                drain(1)
            drain_prereqs()
            for st in range(4 * ns, min(4 * ns + 4, NT)):
                opt_q.append(st)
        # endgame: the hp2 normalization chain (DVE/Pool) has no PE work
        # of its own.  Emit yt0/yt1 partial accumulations for up to four
        # proj units first (they depend only on earlier head-pairs), then
        # finish each with its yt2 step once the final yT mult lands.
        tail_sts = list(opt_q)
        held = []
        for i, st in enumerate(tail_sts[:4]):
            if i % 2 == 0:
                pa = ps.tile([128, 512], F32, tag="mm", bufs=2,
                             name="pa")[:]
                pb = ps.tile([128, 512], F32, tag="mm", bufs=2,
                             name="pb")[:, 0:256]
            else:
                buf = ps.tile([128, 2, 512], F32, tag="sc", bufs=2,
                              name="pt")
                pa, pb = buf[:, 0, :], buf[:, 1, 0:256]
            for yt in range(2):
                nc.tensor.matmul(pa, yT[yt][:, 128 * st:128 * st + 128],
                                 wp1[:, yt, 0:512], start=(yt == 0),
                                 stop=False, skip_group_check=True)
            for yt in range(2):
                nc.tensor.matmul(pb, yT[yt][:, 128 * st:128 * st + 128],
                                 wp1[:, yt, 512:768], start=(yt == 0),
                                 stop=False, skip_group_check=True)
            held.append((st, pa, pb))
        for st, pa, pb in held:
            nc.tensor.matmul(pa, yT[2][:, 128 * st:128 * st + 128],
                             wp1[:, 2, 0:512], start=False, stop=True,
                             skip_group_check=True)
            nc.tensor.matmul(pb, yT[2][:, 128 * st:128 * st + 128],
                             wp1[:, 2, 512:768], start=False, stop=True,
                             skip_group_check=True)
            ot = outp.tile([128, D], F32, tag="ot", name="ot")
            nc.vector.tensor_copy(ot[:, 0:512], pa)
            nc.vector.tensor_copy(ot[:, 512:768], pb)
            nc.sync.dma_start(out=out[128 * st:128 * st + 128, :],
                              in_=ot[:])
        for i, st in enumerate(tail_sts[4:]):
            p4_unit(st, tail=(i % 2 == 1))

    nc.finalize()
    return nc


def shard_inputs(x, w_qkv, b_qkv, w_proj):
    """Host-side sharding: returns list of per-core input dicts."""
    import jax.numpy as jnp

    def bf16(a):
        return jnp.asarray(np.ascontiguousarray(a), dtype=jnp.bfloat16)

    def round_fp32r(a):
        """Round fp32 to fp32r (11 explicit mantissa bits), RNE."""
        a = np.ascontiguousarray(a, dtype=np.float32)
        u = a.view(np.uint32).astype(np.uint64)
        bias = ((u >> 12) & 1) + 0x7FF
        u = ((u + bias) & 0xFFFFF000).astype(np.uint32)
        return u.view(np.float32)

    S = x.shape[1]
    NS = S // 512
    in_maps = []
    for core in range(NCORES):
        b, hg = (core // 2) % x.shape[0], core % 2
        # packed layouts: partition-major so DMA descriptors are multi-KB
        xT_s = (x[b].T.reshape(6, 128, NS, 512)
                .transpose(2, 1, 0, 3))                    # [NS,128,6,512]
        # column 0 carries the q bias: wqkv_s[t*128+p, 0] = bq_pair_t[p]
        bias_col = np.zeros((768, 1), dtype=np.float32)
        cols = [bias_col]
        for p in range(3):
            h0 = 64 * (6 * hg + 2 * p)
            bias_col[128 * p:128 * p + 128, 0] = b_qkv[0:768][h0:h0 + 128]
            cols.append(w_qkv[:, 0:768][:, h0:h0 + 128])      # q pair p
            cols.append(w_qkv[:, 768:1536][:, h0:h0 + 128])   # k pair p
        cols.append(w_qkv[:, 1536:2304][:, 384 * hg:384 * hg + 384])  # v
        wqkv_s = np.concatenate(cols, axis=1)                 # [768, 1153]
        wA = (wqkv_s[:, 0:257]
              .reshape(6, 128, 257).transpose(1, 0, 2))       # [128,6,257]
        wB = (wqkv_s[:, 257:1153]
              .reshape(6, 128, 896).transpose(1, 0, 2))       # [128,6,896]
        wproj_s = (w_proj[384 * hg:384 * hg + 384, :]
                   .reshape(3, 128, 768).transpose(1, 0, 2))  # [128,3,768]
        in_maps.append({
            "xT_s": bf16(xT_s),
            "wqkvA_s": bf16(wA),
            "wqkvB_s": bf16(wB),
            "wproj_s": bf16(wproj_s),
        })
    return in_maps


_CACHED = {}


def _get_program():
    if "nc" not in _CACHED:
        _CACHED["nc"] = build_program()
    return _CACHED["nc"]


def _spot_check(outp, x, w_qkv, b_qkv, w_proj, b_proj):
    """Exact per-row reference on a few rows; returns worst relative error.
    Guards against rare transient bad compiles/executions."""
    B, S, dim = x.shape
    H, HD = 12, 64
    worst = 0.0
    for b in range(B):
        s = min(S - 1, 511 + 512 * b)
        xb = x[b].astype(np.float64)
        q = xb[s] @ w_qkv[:, 0:768] + b_qkv[0:768]
        k = xb[:s + 1] @ w_qkv[:, 768:1536] + b_qkv[768:1536]
        v = xb[:s + 1] @ w_qkv[:, 1536:2304] + b_qkv[1536:2304]
        ys = []
        for h in range(H):
            sc = (k[:, HD * h:HD * h + HD] @ q[HD * h:HD * h + HD]) * 0.125
            e = np.exp(sc - sc.max())
            ys.append((e / e.sum()) @ v[:, HD * h:HD * h + HD])
        row = np.concatenate(ys) @ w_proj + b_proj
        rel = np.abs(outp[b, s] - row).max() / max(np.abs(row).max(), 1e-6)
        worst = max(worst, rel)
    return worst


def kernel(x, w_qkv, b_qkv, w_proj, b_proj):
    import jax
    from concourse.bass_utils import run_bass_kernel_spmd

    x = np.asarray(x, dtype=np.float32)
    w_qkv = np.asarray(w_qkv, dtype=np.float32)
    b_qkv = np.asarray(b_qkv, dtype=np.float32)
    w_proj = np.asarray(w_proj, dtype=np.float32)
    b_proj = np.asarray(b_proj, dtype=np.float32)

    B, S, dim = x.shape
    in_maps = shard_inputs(x, w_qkv, b_qkv, w_proj)
    # v-bias folds out of attention (rows of attn sum to exactly 1):
    # y = attn @ (v + 1 b_v^T) = attn @ v + 1 b_v^T, so its projection is a
    # constant row added on the host along with b_proj.
    bvw = b_qkv[1536:2304] @ w_proj  # [D]
    const_row = (b_proj + bvw)[None, :]

    outp = np.empty((B, S, dim), dtype=np.float32)
    for attempt in range(3):
        nc = _get_program()
        res = run_bass_kernel_spmd(nc, in_maps, core_ids=list(range(NCORES)))
        parts = [m["out_s"] for m in res.results]
        for b in range(B):
            outp[b] = parts[2 * b] + parts[2 * b + 1] + const_row
        if _spot_check(outp, x, w_qkv, b_qkv, w_proj, b_proj) < 5e-3:
            break
        # transient bad build/execution: clear caches, rebuild, rerun
        _CACHED.clear()
        jax.clear_caches()
    return outp


# revision 68
# speedup vs baseline: 1.0108x; 1.0108x over previous
"""Causal self-attention (B=4, S=2048, D=768, H=12) on 8 TRN2 NeuronCores.

Sharding: core = (batch b in 0..3) x (head-group hg in 0..1, 6 heads each).
Host pre-transposes x -> xT per batch (bf16), slices w_qkv columns /
w_proj rows per head-group (bf16), reorders w_qkv columns pair-major
[q_p0|k_p0|q_p1|k_p1|q_p2|k_p2|v].  Each core computes its 6 heads
end-to-end and a partial projection output [S, D]; the host sums the two
head-group partials per batch and adds b_proj plus the (attention-
invariant) v-bias term b_v @ w_proj.  The k bias is dropped entirely:
softmax over k is invariant to the per-q constant (q+bq).bk.

Device design notes:
  - Every matmul's moving operand is BF16 (or f32r with >=256 free), so
    all matmuls run at 1 cycle/row; the PE stream is kept dense so the
    p-state ramps to 2.4 GHz and stays there.
  - Causal masking happens ON the PE: a [128,128] constant matmul
    (lhsT=maskT f32r, rhs=identity bf16) accumulates -240 into the
    upper-triangle of diagonal score blocks before exp(0.125*x).  No
    cross-engine op sits between scores and exp.
  - exp on ACT per (chunk,h) over [128,2,512] PSUM->SBUF bf16; the
    attn@v matmuls lag two chunks behind so ACT latency never stalls
    the in-order PE queue.
  - PSUM->SBUF drains (q bias-add, k copy, v scatter, yh copy, out
    copy) on DVE; normalization broadcast+multiply on Pool; DMA issues
    split across SP/Pool/ACT queues, biggest-stakeholder-first.
  - v carries a ones-column so attn@[v|1] also emits the softmax
    denominator row (yh partition 64).
"""

import numpy as np
from collections import deque
from contextlib import ExitStack

import concourse.bacc as bacc
import concourse.mybir as mybir
from concourse.tile import TileContext

F32 = mybir.dt.float32
F32R = mybir.dt.float32r
BF16 = mybir.dt.bfloat16

D = 768
NCORES = 8
SCALE = 0.125  # 1/sqrt(64)
MASKV = -240.0  # pre-scale mask add: exp(0.125 * -240) = e^-30 ~ 1e-13


def build_program(S=2048):
    NS = S // 512   # q strips
    NT = S // 128   # s tiles
    DT = D // 128   # d tiles (contraction)

    nc = bacc.Bacc()

    # All inputs ship partition-major-packed from the host so every DMA
    # descriptor is a multi-KB contiguous run (descriptor-rate bound
    # otherwise).  wqkv columns: [bias(1) | qp0 kp0 qp1 kp1 qp2 kp2 | v].
    xT = nc.dram_tensor("xT_s", [NS, 128, D // 128, 512], BF16,
                        kind="ExternalInput")
    wqkvA = nc.dram_tensor("wqkvA_s", [128, D // 128, 257], BF16,
                           kind="ExternalInput")
    wqkvB = nc.dram_tensor("wqkvB_s", [128, D // 128, 896], BF16,
                           kind="ExternalInput")
    wproj = nc.dram_tensor("wproj_s", [128, 3, D], BF16,
                           kind="ExternalInput")
    out = nc.dram_tensor("out_s", [S, D], F32, kind="ExternalOutput")

    with TileContext(nc) as tc, ExitStack() as ctx:
        persist = ctx.enter_context(tc.tile_pool(name="persist", bufs=1))

        w1a = persist.tile([128, 6, 257], BF16, tag="w1a", name="w1a")
        w1b = persist.tile([128, 6, 896], BF16, tag="w1b", name="w1b")
        xT1 = persist.tile([128, NS, 6, 512], BF16, tag="xT1", name="xT1")
        wp1 = persist.tile([128, 3, D], BF16, tag="wp1", name="wp1")
        qT = [persist.tile([128, S], BF16, tag=f"qT{i}", name=f"qT{i}")
              for i in range(3)]
        kT = [persist.tile([128, S], BF16, tag=f"kT{i}", name=f"kT{i}")
              for i in range(3)]
        v_sb = [persist.tile([128, 6, 65], BF16, tag=f"v{i}", name=f"v{i}")
                for i in range(NT)]
        yT = [persist.tile([128, S], BF16, tag=f"yT{i}", name=f"yT{i}")
              for i in range(3)]
        bq_sb = persist.tile([128, 3], F32, tag="bq", name="bq_sb")
        mskT_sb = persist.tile([128, 128], BF16, tag="mskT", name="mskT_sb")
        idf = persist.tile([128, 128], F32, tag="idf", name="idf")
        iden_sb = persist.tile([128, 128], BF16, tag="iden", name="iden_sb")
        mkf = persist.tile([128, 128], F32, tag="mkf", name="mkf")

        # identity: ones, zero j<p, zero j>p (affine_select keeps where
        # base + j + channel_multiplier*p  <cmp>  0), then cast to bf16
        nc.gpsimd.memset(idf[:], 1.0)
        nc.gpsimd.affine_select(
            out=idf[:], in_=idf[:],
            compare_op=mybir.AluOpType.is_ge, fill=0.0, base=0,
            pattern=[[1, 128]], channel_multiplier=-1)  # keep j >= p
        nc.gpsimd.affine_select(
            out=idf[:], in_=idf[:],
            compare_op=mybir.AluOpType.is_ge, fill=0.0, base=0,
            pattern=[[-1, 128]], channel_multiplier=1)  # keep p >= j
        nc.vector.tensor_copy(iden_sb[:], idf[:])
        # maskT[r, c] = MASKV where c > r else 0 (built on-device: no DMA
        # on the critical path to the first masked score block)
        nc.gpsimd.memset(mkf[:], 0.0)
        nc.gpsimd.affine_select(
            out=mkf[:], in_=mkf[:],
            compare_op=mybir.AluOpType.is_ge, fill=MASKV, base=0,
            pattern=[[-1, 128]], channel_multiplier=1)  # fill where c > r
        nc.vector.tensor_copy(mskT_sb[:], mkf[:])

        # ---- input DMAs: need-ordered; SP ring carries the first needs,
        # the Pool ring the rest.  All transfers are contiguous-per-
        # partition (multi-KB descriptors).
        nc.scalar.dma_start(out=w1a[:], in_=wqkvA[:])
        nc.sync.dma_start(out=xT1[:, 0], in_=xT[0])
        nc.sync.dma_start(out=w1b[:], in_=wqkvB[:])
        for ns2 in range(1, NS):
            nc.gpsimd.dma_start(out=xT1[:, ns2], in_=xT[ns2])
        nc.gpsimd.dma_start(out=wp1[:], in_=wproj[:])
        for st in range(NT):
            nc.gpsimd.memset(v_sb[st][:, :, 64:65], 1.0)
        # q bias rides in w1a column 0 (bf16); cast once to f32 for the
        # tensor_scalar_add scalar operand
        nc.vector.tensor_copy(
            bq_sb[:], w1a[:, 0:3, 0:1].rearrange("p t c -> p (t c)"))

        ps = ctx.enter_context(tc.tile_pool(name="ps", bufs=1, space="PSUM"))
        expp = ctx.enter_context(tc.tile_pool(name="expp", bufs=8))
        rcp = ctx.enter_context(tc.tile_pool(name="rcp", bufs=4))
        rbp = ctx.enter_context(tc.tile_pool(name="rbp", bufs=2))
        outp = ctx.enter_context(tc.tile_pool(name="outp", bufs=2))

        def pe_touch(ap):
            # Tiny self-matmul so the PE waits on this tile's producer
            # once; later matmuls using it carry at most one sync wait.
            t = ps.tile([1, 1], F32, tag="mm", bufs=2, name="touch")
            nc.tensor.matmul(t[:], ap, ap, start=True, stop=True)

        pe_touch(mskT_sb[:, 0:2].bitcast(F32))
        pe_touch(iden_sb[:, 0:2].bitcast(F32))

        # ---- phase work units (emitted interleaved) ----
        def p1q_unit(ns, p):
            # qT[p][strip ns] = (wq_p.T @ xT) + bq_p   (bf16 out)
            psu = ps.tile([128, 512], F32, tag="mm", bufs=2, name="ps_q")
            for dt_i in range(DT):
                wsl = (w1a[:, dt_i, 1:129] if p == 0 else
                       w1b[:, dt_i, 256 * p - 256:256 * p - 128])
                nc.tensor.matmul(
                    psu[:], wsl, xT1[:, ns, dt_i, :],
                    start=(dt_i == 0), stop=(dt_i == DT - 1))
            nc.vector.tensor_scalar_add(
                qT[p][:, 512 * ns:512 * ns + 512], psu[:],
                bq_sb[:, p:p + 1])

        def p1k_unit(ns, p):
            # kT[p][strip ns] = wk_p.T @ xT   (no bias; bf16 out)
            psu = ps.tile([128, 512], F32, tag="mm", bufs=2, name="ps_k")
            for dt_i in range(DT):
                wsl = (w1a[:, dt_i, 129:257] if p == 0 else
                       w1b[:, dt_i, 256 * p - 128:256 * p])
                nc.tensor.matmul(
                    psu[:], wsl, xT1[:, ns, dt_i, :],
                    start=(dt_i == 0), stop=(dt_i == DT - 1))
            nc.vector.tensor_copy(kT[p][:, 512 * ns:512 * ns + 512], psu[:])

        def p2v_unit(st):
            # v natural for s-tile st (bf16, ones in col 64)
            psu = ps.tile([128, 512], F32, tag="mm", bufs=2, name="ps_v")
            sq, so = st // 4, 128 * (st % 4)
            for dt_i in range(DT):
                nc.tensor.matmul(
                    psu[:, 0:384], xT1[:, sq, dt_i, so:so + 128],
                    w1b[:, dt_i, 512:896],
                    start=(dt_i == 0), stop=(dt_i == DT - 1))
            nc.vector.tensor_copy(
                v_sb[st][:, :, 0:64],
                psu[:, 0:384].rearrange("p (h e) -> p h e", h=6))

        def p4_unit(st, tail=False):
            # partial proj for s-tile st.  In the endgame (attention done)
            # the freed score PSUM banks + the ACT engine double the proj
            # drain pipeline depth.
            if tail:
                buf = ps.tile([128, 2, 512], F32, tag="sc", bufs=2,
                              name="pt")
                pa, pb = buf[:, 0, :], buf[:, 1, 0:256]
            else:
                pa = ps.tile([128, 512], F32, tag="mm", bufs=2, name="pa")[:]
                pb = ps.tile([128, 512], F32, tag="mm", bufs=2,
                             name="pb")[:, 0:256]
            for yt in range(3):
                nc.tensor.matmul(
                    pa, yT[yt][:, 128 * st:128 * st + 128],
                    wp1[:, yt, 0:512], start=(yt == 0), stop=(yt == 2))
            for yt in range(3):
                nc.tensor.matmul(
                    pb, yT[yt][:, 128 * st:128 * st + 128],
                    wp1[:, yt, 512:768], start=(yt == 0), stop=(yt == 2))
            ot = outp.tile([128, D], F32, tag="ot", name="ot")
            nc.vector.tensor_copy(ot[:, 0:512], pa)
            nc.vector.tensor_copy(ot[:, 512:768], pb)
            nc.sync.dma_start(out=out[128 * st:128 * st + 128, :], in_=ot[:])

        pre_q = deque()   # next strip's q/k/v units (due before that strip)
        opt_q = deque()   # proj units (any time after their strip + norm)
        OPT_KEEP = 3      # proj units held back to cover the final tail

        def drain(n):
            for _ in range(n):
                if pre_q:
                    pre_q.popleft()()
                elif len(opt_q) > OPT_KEEP:
                    p4_unit(opt_q.popleft())
                else:
                    return

        def drain_prereqs():
            while pre_q:
                pre_q.popleft()()

        # prologue: strip-0 pair-0 q/k and all strip-0 v before attention;
        # pairs 1,2 emit at their hp boundary (keeps PE fed while ACT ramps)
        p1q_unit(0, 0)
        p1k_unit(0, 0)
        for st in range(4 if NS > 1 else NT):
            p2v_unit(st)
        due_hp = {1: [1], 2: [2]}

        LAG = 2  # attn@v trails the exp pipeline by this many chunks

        # ---- attention ----
        for ns in range(NS):
            if ns + 1 < NS:
                for p in range(3):
                    pre_q.append(lambda a=ns + 1, b=p: p1q_unit(a, b))
                    pre_q.append(lambda a=ns + 1, b=p: p1k_unit(a, b))
                for st in range(4 * (ns + 1), min(4 * (ns + 2), NT)):
                    pre_q.append(lambda a=st: p2v_unit(a))
                if ns == 0:
                    pre_q.append(lambda: pe_touch(wp1[:, 0, 0:2].bitcast(F32)))
            q0 = 512 * ns
            for hp in range(3):
                if ns == 0:
                    for p in due_hp.pop(hp, []):
                        p1q_unit(0, p)
                        p1k_unit(0, p)
                nk = 4 * (ns + 1)
                nchunk = nk // 2
                yh = [ps.tile([65, 512], F32, tag="yh", bufs=2, name="yh0"),
                      ps.tile([65, 512], F32, tag="yh", bufs=2, name="yh1")]

                def emit_att(c, ex_pair):
                    # attn@v accumulation for chunk c (both heads, both u)
                    for h in range(2):
                        for u in range(2):
                            kb = 2 * c + u
                            c0 = max(0, 128 * kb - q0)
                            nc.tensor.matmul(
                                yh[h][:, c0:512],
                                v_sb[kb][:, 2 * hp + h, :],
                                ex_pair[h][:, u, c0:512],
                                start=(kb == 0), stop=(kb == nk - 1),
                                skip_group_check=True)

                pend = deque()
                for c in range(nchunk):
                    ex_pair = []
                    for h in range(2):
                        p0 = 64 * h
                        sc2 = ps.tile([128, 2, 512], F32, tag="sc", bufs=2,
                                      name="sc2")
                        e_lo = 512
                        for u in range(2):
                            kb = 2 * c + u
                            d = kb - 4 * ns
                            q_lo = 128 * d if d >= 0 else 0
                            e_lo = min(e_lo, q_lo)
                            nc.tensor.matmul(
                                sc2[:, u, q_lo:512],
                                kT[hp][p0:p0 + 64, 128 * kb:128 * kb + 128],
                                qT[hp][p0:p0 + 64, q0 + q_lo:q0 + 512],
                                start=True, stop=(d < 0),
                                skip_group_check=True)
                            if d >= 0:
                                nc.tensor.matmul(
                                    sc2[:, u, 128 * d:128 * d + 128],
                                    mskT_sb[:],
                                    iden_sb[:],
                                    start=False, stop=True,
                                    skip_group_check=True)
                        ex2 = expp.tile([128, 2, 512], BF16, tag="exp",
                                        name="ex2")
                        nc.scalar.activation(
                            ex2[:, :, e_lo:512], sc2[:, :, e_lo:512],
                            mybir.ActivationFunctionType.Exp, scale=SCALE)
                        ex_pair.append(ex2)
                    pend.append((c, ex_pair))
                    # front-load pre-strip fillers so their DVE drains land
                    # well before the next strip's scores need the tiles
                    if c >= 1 or (ns == 0 and hp == 0):
                        drain(2 if pre_q else 1)
                    if len(pend) > LAG:
                        emit_att(*pend.popleft())
                while pend:
                    emit_att(*pend.popleft())
                    drain(1)

                # tail: free yh banks and normalize yT.  Mid-kernel the yh
                # copies go first (they free the banks for the next head
                # pair); on the very last tail the reciprocal path leads so
                # the Pool broadcasts start as early as possible (the final
                # proj units' yt2 step waits on the mult).
                last_tail = (ns == NS - 1 and hp == 2)
                for h in range(2):
                    ys = yT[hp][64 * h:64 * h + 64, q0:q0 + 512]
                    lrow = rcp.tile([1, 512], F32, tag="lrow", name="lrow")
                    rec = rcp.tile([1, 512], F32, tag="rec", name="rec")
                    rb = rbp.tile([128, 512], F32, tag="rb", name="rb")
                    if last_tail:
                        nc.vector.tensor_copy(lrow[:], yh[h][64:65, :])
                        nc.vector.reciprocal_approx_fast(rec[:], lrow[:])
                        nc.gpsimd.partition_broadcast(rb[:], rec[:])
                        nc.vector.tensor_copy(ys, yh[h][0:64, :])
                    else:
                        nc.vector.tensor_copy(ys, yh[h][0:64, :])
                        nc.vector.tensor_copy(lrow[:], yh[h][64:65, :])
                        nc.vector.reciprocal_approx_fast(rec[:], lrow[:])
                        nc.gpsimd.partition_broadcast(rb[:], rec[:])
                    nc.vector.tensor_mul(ys, ys, rb[64 * h:64 * h + 64, :])
                drain(1)
            drain_prereqs()
            for st in range(4 * ns, min(4 * ns + 4, NT)):
                opt_q.append(st)
        # endgame: the hp2 normalization chain (DVE/Pool) has no PE work
        # of its own.  Emit yt0/yt1 partial accumulations for up to four
        # proj units first (they depend only on earlier head-pairs), then
        # finish each with its yt2 step once the final yT mult lands.
        tail_sts = list(opt_q)
        held = []
        for i, st in enumerate(tail_sts[:4]):
            if i % 2 == 0:
                pa = ps.tile([128, 512], F32, tag="mm", bufs=2,
                             name="pa")[:]
                pb = ps.tile([128, 512], F32, tag="mm", bufs=2,
                             name="pb")[:, 0:256]
            else:
                buf = ps.tile([128, 2, 512], F32, tag="sc", bufs=2,
                              name="pt")
                pa, pb = buf[:, 0, :], buf[:, 1, 0:256]
            for yt in range(2):
                nc.tensor.matmul(pa, yT[yt][:, 128 * st:128 * st + 128],
                                 wp1[:, yt, 0:512], start=(yt == 0),
                                 stop=False, skip_group_check=True)
            for yt in range(2):
                nc.tensor.matmul(pb, yT[yt][:, 128 * st:128 * st + 128],
                                 wp1[:, yt, 512:768], start=(yt == 0),
                                 stop=False, skip_group_check=True)
            held.append((st, pa, pb))
        for st, pa, pb in held:
            nc.tensor.matmul(pa, yT[2][:, 128 * st:128 * st + 128],
                             wp1[:, 2, 0:512], start=False, stop=True,
                             skip_group_check=True)
            nc.tensor.matmul(pb, yT[2][:, 128 * st:128 * st + 128],
                             wp1[:, 2, 512:768], start=False, stop=True,
                             skip_group_check=True)
            ot = outp.tile([128, D], F32, tag="ot", name="ot")
            nc.vector.tensor_copy(ot[:, 0:512], pa)
            nc.vector.tensor_copy(ot[:, 512:768], pb)
            nc.sync.dma_start(out=out[128 * st:128 * st + 128, :],
                              in_=ot[:])
        for i, st in enumerate(tail_sts[4:]):
            p4_unit(st, tail=(i % 2 == 1))

    nc.finalize()
    return nc


def shard_inputs(x, w_qkv, b_qkv, w_proj):
    """Host-side sharding: returns list of per-core input dicts."""
    import jax.numpy as jnp

    def bf16(a):
        return jnp.asarray(np.ascontiguousarray(a), dtype=jnp.bfloat16)

    def round_fp32r(a):
        """Round fp32 to fp32r (11 explicit mantissa bits), RNE."""
        a = np.ascontiguousarray(a, dtype=np.float32)
        u = a.view(np.uint32).astype(np.uint64)
        bias = ((u >> 12) & 1) + 0x7FF
        u = ((u + bias) & 0xFFFFF000).astype(np.uint32)
        return u.view(np.float32)

    S = x.shape[1]
    NS = S // 512
    in_maps = []
    for core in range(NCORES):
        b, hg = (core // 2) % x.shape[0], core % 2
        # packed layouts: partition-major so DMA descriptors are multi-KB
        xT_s = (x[b].T.reshape(6, 128, NS, 512)
                .transpose(2, 1, 0, 3))                    # [NS,128,6,512]
        # column 0 carries the q bias: wqkv_s[t*128+p, 0] = bq_pair_t[p]
        bias_col = np.zeros((768, 1), dtype=np.float32)
        cols = [bias_col]
        for p in range(3):
            h0 = 64 * (6 * hg + 2 * p)
            bias_col[128 * p:128 * p + 128, 0] = b_qkv[0:768][h0:h0 + 128]
            cols.append(w_qkv[:, 0:768][:, h0:h0 + 128])      # q pair p
            cols.append(w_qkv[:, 768:1536][:, h0:h0 + 128])   # k pair p
        cols.append(w_qkv[:, 1536:2304][:, 384 * hg:384 * hg + 384])  # v
        wqkv_s = np.concatenate(cols, axis=1)                 # [768, 1153]
        wA = (wqkv_s[:, 0:257]
              .reshape(6, 128, 257).transpose(1, 0, 2))       # [128,6,257]
        wB = (wqkv_s[:, 257:1153]
              .reshape(6, 128, 896).transpose(1, 0, 2))       # [128,6,896]
        wproj_s = (w_proj[384 * hg:384 * hg + 384, :]
                   .reshape(3, 128, 768).transpose(1, 0, 2))  # [128,3,768]
        in_maps.append({
            "xT_s": bf16(xT_s),
            "wqkvA_s": bf16(wA),
            "wqkvB_s": bf16(wB),
            "wproj_s": bf16(wproj_s),
        })
    return in_maps


_CACHED = {}


def _get_program():
    if "nc" not in _CACHED:
        _CACHED["nc"] = build_program()
    return _CACHED["nc"]


def _spot_check(outp, x, w_qkv, b_qkv, w_proj, b_proj):
    """Exact per-row reference on a few rows; returns worst relative error.
    Guards against rare transient bad compiles/executions."""
    B, S, dim = x.shape
    H, HD = 12, 64
    worst = 0.0
    for b in range(B):
        s = min(S - 1, 511 + 512 * b)
        xb = x[b].astype(np.float64)
        q = xb[s] @ w_qkv[:, 0:768] + b_qkv[0:768]
        k = xb[:s + 1] @ w_qkv[:, 768:1536] + b_qkv[768:1536]
        v = xb[:s + 1] @ w_qkv[:, 1536:2304] + b_qkv[1536:2304]
        ys = []
        for h in range(H):
            sc = (k[:, HD * h:HD * h + HD] @ q[HD * h:HD * h + HD]) * 0.125
            e = np.exp(sc - sc.max())
            ys.append((e / e.sum()) @ v[:, HD * h:HD * h + HD])
        row = np.concatenate(ys) @ w_proj + b_proj
        rel = np.abs(outp[b, s] - row).max() / max(np.abs(row).max(), 1e-6)
        worst = max(worst, rel)
    return worst


def kernel(x, w_qkv, b_qkv, w_proj, b_proj):
    import jax
    from concourse.bass_utils import run_bass_kernel_spmd

    x = np.asarray(x, dtype=np.float32)
    w_qkv = np.asarray(w_qkv, dtype=np.float32)
    b_qkv = np.asarray(b_qkv, dtype=np.float32)
    w_proj = np.asarray(w_proj, dtype=np.float32)
    b_proj = np.asarray(b_proj, dtype=np.float32)

    B, S, dim = x.shape
    in_maps = shard_inputs(x, w_qkv, b_qkv, w_proj)
    # v-bias folds out of attention (rows of attn sum to exactly 1):
    # y = attn @ (v + 1 b_v^T) = attn @ v + 1 b_v^T, so its projection is a
    # constant row added on the host along with b_proj.
    bvw = b_qkv[1536:2304] @ w_proj  # [D]
    const_row = (b_proj + bvw)[None, :]

    outp = np.empty((B, S, dim), dtype=np.float32)
    for attempt in range(3):
        nc = _get_program()
        res = run_bass_kernel_spmd(nc, in_maps, core_ids=list(range(NCORES)))
        parts = [m["out_s"] for m in res.results]
        for b in range(B):
            outp[b] = parts[2 * b] + parts[2 * b + 1] + const_row
        if _spot_check(outp, x, w_qkv, b_qkv, w_proj, b_proj) < 5e-3:
            break
        # transient bad build/execution: clear caches, rebuild, rerun
        _CACHED.clear()
        jax.clear_caches()
    return outp
